# revision 32
# baseline (speedup 1.0000x reference)
"""LoRA self-attention processor on 8 TRN2 NeuronCores.

Problem: B=4, S=2048, D=640, H=8 heads (hd=80), LoRA rank 4.
  q/k/v = x @ (W + up@down).T ; per-head attention; out = attn @ (Wo + o_up@o_down).T + bo

Sharding: batch*head parallel. Core c -> batch b=c//2, head-group g=c%2
(4 heads). Host folds the rank-4 LoRA updates into the weights (exact
algebra) and pre-transposes/casts operands to bf16 (fp16 matmuls are
~4x slower on TRN2 hardware despite what the cost model says).

Per-core structure (PSUM-accumulated fp32):
  v/k projections up front: v as [128s, 4, 80] -> SBUF [128, 4, 81]
  bf16 with an appended ones column (softmax denominator trick); k as
  [80, S] per head. q projections are emitted inside the attention
  loop where the PE otherwise idles behind ACT. All stationary matmul
  operands are 128 columns (w_qk host-padded to 688 cols; overshooting
  head slices is harmless since only rows 0:80 of PSUM are copied out)
  so FWL can hide every weight load.

  Attention per (chunk c of 512 q, head h): 16 score matmuls
  (k-position-major [128k, 512q]) grouped into 8x2 k-tiles per PSUM
  allocation ([128, 1024] = 2 banks, bufs=3 = 6 banks: reuse distance
  3 gives the exp consumers 2.5us of slack so score matmuls never
  stall on the slot rotation); exp per group with the exact power-of-2
  descale 1/256 (256*sm_scale folded into wq on host). Exp is SPLIT
  between ACT (func Exp) and DVE (Schraudolph bf16 bit-trick via
  tensor_scalar into a u16 bitcast, rel err contribution ~4e-3):
  per-iteration DVE share picked so both ACT and DVE stay under that
  iteration's PE time (first half 2/8 groups, light second-half iters
  3/8, out-carrying iters 0). PV with probs as stationary [128,128]
  tiles (FWL) producing attn for all 4 q-tiles in ONE [128, 4, 81]
  psum tile (1 "at" allocation/iter, not 4); column 80 is the
  denominator: one strided DVE reciprocal + per-partition
  tensor_scalar mult normalizes to bf16. PE transpose (via identity)
  back to [80, q], scattered into three [128, S] tiles packing heads
  at 96-row offsets (pads zeroed via DMA) so the output projection
  contracts as 3 full 128-row tiles instead of 4x80. Out-projection
  d-units are DELAYED TWO iterations and interleaved one-per-score-
  group (the out matmuls pad PE between score bursts so ACT keeps up);
  staged into one [128, 5, 512] SBUF tile, stored with 1 fused DMA
  (per-d for the last chunk to pipeline the epilogue). Input DMAs are
  fused 3D-AP loads (one per tensor / xT column chunk) since each SP
  DMA issue costs ~565ns. In the drain (last iters, no scores to
  overlap) normalize/scatter/stage copies alternate DVE/ACT.
  Partial out written fp32; host sums the two core-partials per
  batch + bias.

Engine balance (sim): PE ~152us busy (89.9% occupancy), ACT ~104us,
DVE ~108us, SP(DMA) 36us. Iteration order head-pair-blocked ((h0,h1)
over all chunks, then (h2,h3)): v and k0/k1 project up front
(interleaved with xT chunk DMA arrival), k2/k3 stream during the
first half, q one iteration ahead. PSUM budget exactly 8 banks
(scores 6, shared qproj/kproj/PV/transpose/out 2).
Measured 170.5us/iter on HW vs 188.2us for the previous baseline,
interleaved A/B on the same device session (single-shot measurements
swing +/-7% with co-tenancy; use compare.py). Sim predicts 169.1.
"""
import numpy as np
import ml_dtypes

B, S, D, H, HD, R = 4, 2048, 640, 8, 80, 4
HPC = H // 2          # heads per core
GDIM = HPC * HD       # 320 head-dims per core
NCORES = 8
NKT = S // 128        # 16 key tiles
NQC = S // 512        # 4 query chunks
NCT = D // 128        # 5 contraction tiles
NGG = NKT // 2        # 8 exp groups of 2 k-tiles
SM_SCALE = 1.0 / float(np.sqrt(HD))
FOLD_Q = 256.0 * SM_SCALE          # folded into wq on host
EXP_SCALE = 1.0 / 256.0            # ACT descale (power of 2: exact)
SCH_MUL = float((128.0 / np.log(2.0)) / 256.0)
SCH_ADD = 16250.5                  # bf16 exp bias + minimax shift
# Exp groups computed on DVE via Schraudolph bf16-bit trick instead of ACT
# exp: rebalances ACT (was the co-bottleneck at 8.6us/iter vs PE 8.96) down
# to ~5.7us/iter. HW-verified rel err 9.9e-3 at a similar DVE share.
DVE_GROUPS = frozenset((5, 6, 7))

import os
KMODE = os.environ.get("KMODE", "full")  # bisect: proj | noschrau | full
if KMODE == "noschrau":
    DVE_GROUPS = frozenset()
elif KMODE.startswith("dve"):
    DVE_GROUPS = frozenset(int(ch) for ch in KMODE[3:])
SCH_U16TILE = os.environ.get("SCH_U16TILE", "0") == "1"
QKDT = os.environ.get("QKDT", "bf16")  # fp16 | bf16 for x/wqk/wv/qk_sb
TPMODE = os.environ.get("TP", "pe")  # pe | dma attnT transpose path
# bisect flags for the structural changes vs the 187us baseline
SGRP8 = os.environ.get("SGRP", "8") == "8"   # 8x2 score groups vs 4x3+2x2
OUTI = os.environ.get("OUTI", "1") == "1"    # delayed+interleaved out-units
DMAF = os.environ.get("DMAF", "1") == "1"    # fused 3D DMAs vs per-tile

_cache = {}


def _body(tc, xT, w_qk, w_v, w_o, zpad, idm, outT):
    import concourse.mybir as mybir

    nc = tc.nc
    bf = mybir.dt.bfloat16
    f16 = mybir.dt.float16 if QKDT == "fp16" else mybir.dt.bfloat16
    f32 = mybir.dt.float32
    u16 = mybir.dt.uint16
    Exp = mybir.ActivationFunctionType.Exp
    ActCopy = mybir.ActivationFunctionType.Copy
    Add = mybir.AluOpType.add
    Mult = mybir.AluOpType.mult

    # alternate psum->sbuf copies between ACT and DVE
    flip = [0]

    def copy_alt(out, in_):
        flip[0] ^= 1
        if flip[0]:
            nc.scalar.copy(out=out, in_=in_)
        else:
            nc.vector.tensor_copy(out=out, in_=in_)

    with tc.tile_pool(name="weights", bufs=1) as wpool, \
         tc.tile_pool(name="persist", bufs=1) as pers:
        # Fused DMAs (one InstDMACopy per logical tensor / xT column chunk):
        # the SP sequencer costs ~565ns per DMA issue, so many small loads
        # serialize the prologue. Order: wv + xT-c0 first (unblocks the
        # first v-projections ASAP), then wqk (k-proj), remaining xT.
        if DMAF:
            xTall = pers.tile([128, NCT, S], f16, name="xTa", tag="xTa")
            xT_t = [xTall[:, i, :] for i in range(NCT)]
            xTv = xT.rearrange("(i p) c -> p i c", i=NCT)
            nc.sync.dma_start(out=xTall[:, :, 0:512], in_=xTv[:, :, 0:512])
            wvall = wpool.tile([128, NCT, GDIM], f16, name="wva", tag="wva")
            nc.sync.dma_start(out=wvall,
                              in_=w_v.rearrange("(i p) c -> p i c", i=NCT))
            wv_t = [wvall[:, i, :] for i in range(NCT)]
            wqkall = wpool.tile([128, NCT, 2 * GDIM + 48], f16, name="wqka",
                                tag="wqka")
            nc.sync.dma_start(out=wqkall,
                              in_=w_qk.rearrange("(i p) c -> p i c", i=NCT))
            wqk_t = [wqkall[:, i, :] for i in range(NCT)]
            nc.sync.dma_start(out=xTall[:, :, 512:1024],
                              in_=xTv[:, :, 512:1024])
            nc.sync.dma_start(out=xTall[:, :, 1024:2048],
                              in_=xTv[:, :, 1024:2048])
            woall = wpool.tile([128, 3, D], bf, name="woa", tag="woa")
            nc.sync.dma_start(out=woall,
                              in_=w_o.rearrange("(i p) c -> p i c", i=3))
            wo_t = [woall[:, i, :] for i in range(3)]
        else:
            xT_t = []
            for i in range(NCT):
                t = pers.tile([128, S], f16, name=f"xT{i}", tag=f"xT{i}")
                xT_t.append(t)
            for i in range(NCT):
                nc.sync.dma_start(out=xT_t[i][:, 0:1024],
                                  in_=xT[128 * i:128 * (i + 1), 0:1024])
            wv_t = []
            for i in range(NCT):
                t = wpool.tile([128, GDIM], f16, name=f"wv{i}", tag=f"wv{i}")
                nc.sync.dma_start(out=t, in_=w_v[128 * i:128 * (i + 1), :])
                wv_t.append(t)
            wqk_t = []
            for i in range(NCT):
                t = wpool.tile([128, 2 * GDIM + 48], f16, name=f"wqk{i}",
                               tag=f"wqk{i}")
                nc.sync.dma_start(out=t, in_=w_qk[128 * i:128 * (i + 1), :])
                wqk_t.append(t)
            for i in range(NCT):
                nc.sync.dma_start(out=xT_t[i][:, 1024:2048],
                                  in_=xT[128 * i:128 * (i + 1), 1024:2048])
            wo_t = []
            for i in range(3):
                t = wpool.tile([128, D], bf, name=f"wo{i}", tag=f"wo{i}")
                nc.sync.dma_start(out=t, in_=w_o[128 * i:128 * (i + 1), :])
                wo_t.append(t)
        idt = wpool.tile([128, 128], bf, name="idt", tag="idt")
        nc.sync.dma_start(out=idt, in_=idm)

        qk_sb = [pers.tile([HD, S], f16, name=f"qkT{i}", tag=f"qkT{i}")
                 for i in range(2 * HPC)]
        vcomb = [pers.tile([128, HPC, HD + 1], bf, name=f"vc{s}", tag=f"vc{s}")
                 for s in range(NKT)]
        for s in range(NKT):
            nc.gpsimd.memset(vcomb[s][:, :, HD:HD + 1], 1.0)
        attnT = [pers.tile([128, S], bf, name=f"anp{i}", tag=f"anp{i}")
                 for i in range(3)]
        # zero the four 16-row pad strips (96h+80 .. 96h+96) via DMA
        for h in range(HPC):
            t, r = divmod(96 * h + HD, 128)
            nc.sync.dma_start(out=attnT[t][r:r + 16, :], in_=zpad)

        # ---------------- v/k projections ----------------
        # Interleaved by xT chunk arrival: v s-tiles for chunk c as soon as
        # xT[:, c] lands, k(h0) for that chunk right after (needs wqk too).
        with tc.tile_pool(name="pjps", bufs=3, space="PSUM") as pjps:
            def emit_vproj(s):
                # v projection: [128s, 4, 80], lhsT = xT (128 cols -> FWL)
                pv = pjps.tile([128, HPC, HD], f32, name="vps", tag="vps")
                for k in range(NCT):
                    nc.tensor.matmul(pv, xT_t[k][:, 128 * s:128 * (s + 1)],
                                     wv_t[k], start=(k == 0),
                                     stop=(k == NCT - 1))
                nc.vector.tensor_copy(out=vcomb[s][:, :, 0:HD], in_=pv)

            def emit_kproj0(hh, c):
                cs = slice(512 * c, 512 * (c + 1))
                ps_ = pjps.tile([128, 512], f32, name="qkps", tag="qkps")
                for k in range(NCT):
                    nc.tensor.matmul(
                        ps_, wqk_t[k][:, HD * hh:HD * hh + 128], xT_t[k][:, cs],
                        start=(k == 0), stop=(k == NCT - 1))
                nc.vector.tensor_copy(out=qk_sb[hh][:, cs], in_=ps_[0:HD, :])

            # k projection here for heads 0/1 only; k for heads 2/3 and all
            # q are projected inside the attention pipeline (head-outer
            # iteration) where PE otherwise idles behind ACT
            for c in range(NQC):
                for s in range(4 * c, 4 * (c + 1)):
                    emit_vproj(s)
                emit_kproj0(HPC, c)
            for c in range(NQC):
                emit_kproj0(HPC + 1, c)

        # ---------------- attention + out projection ----------------
        if KMODE == "proj":
            return
        with tc.tile_pool(name="scps", bufs=3 if SGRP8 else 2,
                          space="PSUM") as scps, \
             tc.tile_pool(name="atps", bufs=2, space="PSUM") as atps, \
             tc.tile_pool(name="probs", bufs=2) as prpool, \
             tc.tile_pool(name="anp", bufs=12) as anpool, \
             tc.tile_pool(name="tstg", bufs=3) as tstg, \
             tc.tile_pool(name="rpp", bufs=12) as rppool, \
             tc.tile_pool(name="obp", bufs=2) as obpool:

            iters = [(c, h) for hb in (0, 1) for c in range(NQC)
                     for h in (2 * hb, 2 * hb + 1)]
            if SGRP8:
                GRP = [2] * 8          # k-tiles per exp group (sums to 16)
                GOFF = [0, 2, 4, 6, 8, 10, 12, 14]
                SCPAD = 1024
            else:
                GRP = [3, 3, 3, 3, 2, 2]
                GOFF = [0, 3, 6, 9, 12, 14]
                SCPAD = 1536
            pbs_cur, pbs_prev = [None] * NKT, [None] * NKT

            def emit_qproj(c, h):
                # q projection for (h, c), psum slot shared with out-proj
                cs = slice(512 * c, 512 * (c + 1))
                ps_ = atps.tile([128, 512], f32, name="qps", tag="at")
                for k in range(NCT):
                    nc.tensor.matmul(
                        ps_, wqk_t[k][:, HD * h:HD * h + 128], xT_t[k][:, cs],
                        start=(k == 0), stop=(k == NCT - 1))
                nc.vector.tensor_copy(out=qk_sb[h][:, cs], in_=ps_[0:HD, :])

            def emit_kproj(c, h):
                # k projection for head h, chunk c (streamed one head ahead)
                hh = HPC + h
                cs = slice(512 * c, 512 * (c + 1))
                ps_ = atps.tile([128, 512], f32, name="kps", tag="at")
                for k in range(NCT):
                    nc.tensor.matmul(
                        ps_, wqk_t[k][:, HD * hh:HD * hh + 128], xT_t[k][:, cs],
                        start=(k == 0), stop=(k == NCT - 1))
                nc.vector.tensor_copy(out=qk_sb[hh][:, cs], in_=ps_[0:HD, :])

            def emit_score_group(c, h, kk, use_dve=True):
                cs = slice(512 * c, 512 * (c + 1))
                n = GRP[kk]
                sc = scps.tile([128, 512 * n], f32, name="sc", tag="sc",
                               padded_shape=[128, SCPAD])
                for p in range(n):
                    k = GOFF[kk] + p
                    nc.tensor.matmul(sc[:, 512 * p:512 * (p + 1)],
                                     qk_sb[HPC + h][:, 128 * k:128 * (k + 1)],
                                     qk_sb[h][:, cs], start=True, stop=True)
                pb = prpool.tile([128, 512 * n], bf, name="pb", tag=f"pb{kk}",
                                 padded_shape=[128, SCPAD])
                if kk in DVE_GROUPS and use_dve:
                    nc.vector.tensor_scalar(
                        out=pb.bitcast(u16), in0=sc, scalar1=SCH_MUL,
                        scalar2=SCH_ADD, op0=Mult, op1=Add)
                else:
                    nc.scalar.activation(out=pb, in_=sc, func=Exp,
                                         scale=EXP_SCALE)
                for p in range(n):
                    pbs_cur[GOFF[kk] + p] = pb[:, 512 * p:512 * (p + 1)]

            def emit_pv(c, h, drain=False):
                # attn [128q, 81] for all 4 q-tiles of chunk c in ONE psum
                # tile [128, 4, 81] (1296B = 1 bank slot): 1 "at" allocation
                # per iter instead of 4, cutting PSUM slot-reuse stalls.
                ap4 = atps.tile([128, 4, HD + 1], f32, name="ap4", tag="at")
                for j in range(4):
                    for k in range(NKT):
                        lhsT = pbs_prev[k][:, 128 * j:128 * (j + 1)]
                        nc.tensor.matmul(ap4[:, j, :], lhsT,
                                         vcomb[k][:, h:h + 1, :],
                                         start=(k == 0), stop=(k == NKT - 1))
                rp4 = rppool.tile([128, 4], f32, name="rp4", tag="rp")
                nc.vector.reciprocal(out=rp4, in_=ap4[:, :, HD:HD + 1])
                ans = []
                for j in range(4):
                    an = anpool.tile([128, 128], bf, name="an", tag="an")
                    if drain and j % 2 == 1:
                        # drain: no scores to overlap, split the normalize
                        # chain ACT/DVE so the transposes unblock sooner
                        nc.scalar.activation(out=an[:, 0:HD],
                                             in_=ap4[:, j, 0:HD],
                                             func=ActCopy,
                                             scale=rp4[:, j:j + 1])
                    else:
                        nc.vector.tensor_scalar(out=an[:, 0:HD],
                                                in0=ap4[:, j, 0:HD],
                                                scalar1=rp4[:, j:j + 1],
                                                scalar2=None, op0=Mult)
                    ans.append(an)
                return ans

            def emit_transposes(c, h, ans, drain=False):
                cs = slice(512 * c, 512 * (c + 1))
                if TPMODE == "dma":
                    tp = tstg.tile([128, 512], bf, name="tp", tag="tstg")
                    for j in range(4):
                        nc.sync.dma_start_transpose(
                            out=tp[:, 128 * j:128 * (j + 1)], in_=ans[j])
                else:
                    tp = atps.tile([128, 512], bf, name="tp", tag="at")
                    for j in range(4):
                        nc.tensor.matmul(tp[:, 128 * j:128 * (j + 1)], ans[j],
                                         idt, is_transpose=True,
                                         start=True, stop=True)
                # scatter rows into the packed attnT at offset 96h.
                # BIR: non-zero base partition allows <= 32 partitions per AP,
                # so emit 32-row chunks (all bases stay 32-aligned).
                # In the drain (no scores to overlap), alternate the copies
                # between DVE and ACT to halve the serial chain.
                base = 96 * h
                r = 0
                ci = 0
                while r < HD:
                    t, off = divmod(base + r, 128)
                    # base-0 APs may span any partition count; others max 32
                    n = min(HD - r, 128 - off) if (off == 0 and r == 0) \
                        else min(32, HD - r, 128 - off)
                    if drain and ci % 2 == 1:
                        nc.scalar.copy(out=attnT[t][off:off + n, cs],
                                       in_=tp[r:r + n, :])
                    else:
                        nc.vector.tensor_copy(out=attnT[t][off:off + n, cs],
                                              in_=tp[r:r + n, :])
                    ci += 1
                    r += n

            outTv = outT.rearrange("(i p) c -> p i c", i=NCT)

            def emit_out_units(c, fused=True, drain=False):
                # 5 d-units, interleaved by the caller with score groups so
                # ACT gets drain time between score matmul bursts. Fused
                # store: 5 d-tiles staged into one SBUF tile, 1 DMA. The
                # last chunk stores per-d so the epilogue DMA pipelines with
                # the staging copies instead of waiting for all 5.
                cs = slice(512 * c, 512 * (c + 1))
                ob = obpool.tile([128, NCT, 512], f32, name="ob", tag="ob")

                def unit(d):
                    def go():
                        op = atps.tile([128, 512], f32, name="op", tag="at")
                        for i in range(3):
                            nc.tensor.matmul(
                                op, wo_t[i][:, 128 * d:128 * (d + 1)],
                                attnT[i][:, cs], start=(i == 0), stop=(i == 2))
                        if drain and d % 2 == 1:
                            nc.scalar.copy(out=ob[:, d, :], in_=op)
                        else:
                            nc.vector.tensor_copy(out=ob[:, d, :], in_=op)
                        if not fused:
                            nc.sync.dma_start(out=outTv[:, d, cs],
                                              in_=ob[:, d, :])
                        elif d == NCT - 1:
                            nc.sync.dma_start(out=outTv[:, :, cs], in_=ob)
                    return go
                return [unit(d) for d in range(NCT)]

            # software pipeline: q-proj of iter i+1, scores of iter i
            # (interleaved with the out-proj d-units of the chunk finished
            # two iters ago - the out matmuls pad PE between score groups so
            # ACT/exp keeps up with the scps slot rotation), PV/transposes
            # of iter i-1. Per-iteration DVE exp share picked to keep both
            # ACT and DVE under that iteration's PE time: iters carrying
            # out-units run exps ACT-only (DVE does the staging copies, PE
            # has the extra out matmuls), light second-half iters shift 3
            # groups to DVE, first-half iters (q/k-proj heavy on PE) 2.
            pending_out = None
            for i in range(len(iters) + 2):
                cur = iters[i] if i < len(iters) else None
                prev = iters[i - 1] if 1 <= i <= len(iters) else None
                drain = cur is None
                units = []
                if pending_out is not None:
                    units = emit_out_units(*pending_out, drain=drain)
                    pending_out = None
                if i == 0:
                    emit_qproj(*iters[0])  # prime: q for the first iter
                if i + 1 < len(iters):
                    emit_qproj(*iters[i + 1])
                if cur is not None and cur[1] < 2:
                    # k for heads 2/3, chunk c, consumed in the second half
                    emit_kproj(cur[0], cur[1] + 2)
                if cur is not None:
                    n_dve = 0 if units else (2 if i < 8 else 3)
                    for kk in range(len(GRP)):
                        emit_score_group(cur[0], cur[1], kk,
                                         use_dve=(kk >= len(GRP) - n_dve))
                        if kk < len(units):
                            units[kk]()
                else:
                    for u in units:
                        u()
                if prev is not None:
                    ans = emit_pv(prev[0], prev[1])
                    emit_transposes(prev[0], prev[1], ans, drain=drain)
                    if prev[1] == HPC - 1:
                        pending_out = (prev[0], prev[0] != NQC - 1)
                pbs_cur, pbs_prev = [None] * NKT, pbs_cur


def build_nc(loop=1):
    import concourse.mybir as mybir
    import concourse.tile as tile
    from concourse import bacc

    bf = mybir.dt.bfloat16
    f16 = mybir.dt.float16 if QKDT == "fp16" else mybir.dt.bfloat16
    f32 = mybir.dt.float32
    nc = bacc.Bacc("TRN2", target_bir_lowering=False, debug=False,
                   num_devices=NCORES)
    xT = nc.dram_tensor("xT", [D, S], f16, kind="ExternalInput").ap()
    w_qk = nc.dram_tensor("w_qk", [D, 2 * GDIM + 48], f16, kind="ExternalInput").ap()
    w_v = nc.dram_tensor("w_v", [D, GDIM], f16, kind="ExternalInput").ap()
    w_o = nc.dram_tensor("w_o", [3 * 128, D], bf, kind="ExternalInput").ap()
    zpad = nc.dram_tensor("zpad", [16, S], bf, kind="ExternalInput").ap()
    idm = nc.dram_tensor("idm", [128, 128], bf, kind="ExternalInput").ap()
    outT = nc.dram_tensor("outT", [D, S], f32, kind="ExternalOutput").ap()
    with tile.TileContext(nc) as tc:
        if loop == 1:
            _body(tc, xT, w_qk, w_v, w_o, zpad, idm, outT)
        else:
            with tc.For_i(0, loop, 1):
                _body(tc, xT, w_qk, w_v, w_o, zpad, idm, outT)
    nc.compile()
    return nc


def make_in_maps(inputs):
    """Host-side shard + layout prep. inputs: full-size fp32 arrays."""
    f = {k: np.asarray(v, dtype=np.float64) for k, v in inputs.items()}
    w_eff = {}
    for nm in ("q", "k", "v", "o"):
        w_eff[nm] = (f[f"w{nm}"] + f[f"{nm}_up"] @ f[f"{nm}_down"])
    bfd = ml_dtypes.bfloat16
    hdt = np.float16 if QKDT == "fp16" else bfd
    x = f["hidden_states"]  # [B, S, D]
    idm = np.eye(128, dtype=bfd)
    in_maps = []
    for c in range(NCORES):
        b, g = divmod(c, 2)
        rows = slice(GDIM * g, GDIM * (g + 1))
        xT = np.ascontiguousarray(x[b].T).astype(hdt)
        wq = (w_eff["q"][rows, :] * FOLD_Q).T  # [640, 320], descaled in exp
        wk = w_eff["k"][rows, :].T
        w_qk = np.ascontiguousarray(np.concatenate(
            [wq, wk, np.zeros((D, 48))], axis=1)).astype(hdt)
        w_v = np.ascontiguousarray(w_eff["v"][rows, :].T).astype(hdt)
        wo_rows = w_eff["o"][:, rows].T  # [320, 640]
        w_o = np.zeros((384, 640), np.float64)
        for h in range(HPC):
            w_o[96 * h:96 * h + HD] = wo_rows[HD * h:HD * (h + 1)]
        w_o = np.ascontiguousarray(w_o).astype(bfd)
        zp = np.zeros((16, S), bfd)
        in_maps.append({"xT": xT, "w_qk": w_qk, "w_v": w_v, "w_o": w_o,
                        "zpad": zp, "idm": idm})
    return in_maps


def assemble_out(results, bo):
    out = np.empty((B, S, D), np.float32)
    for b in range(B):
        pt = results[2 * b]["outT"] + results[2 * b + 1]["outT"]  # [640, 2048]
        out[b] = pt.T + bo[None, :].astype(np.float32)
    return out


def kernel(**inputs):
    from concourse.bass_utils import run_bass_kernel_spmd

    if "nc" not in _cache:
        _cache["nc"] = build_nc()
    nc = _cache["nc"]
    in_maps = make_in_maps(inputs)
    res = run_bass_kernel_spmd(nc, in_maps, list(range(NCORES)))
    return assemble_out(res.results, np.asarray(inputs["bo"], np.float32))

